# revision 25
# baseline (speedup 1.0000x reference)
"""Trainium2 Bass kernel for nn_AdditiveAttention (B=8, Q=512, K=1024, D=128, H=64).

Strategy: data-parallel over batch (1 batch element per NeuronCore, 8 cores).

Per-core math (q in [0,512), k in [0,1024), h in [0,64)):
    qh = queries @ W_q            [Q, H]
    kh = keys @ W_k               [K, H]
    scores[q, k] = sum_h w_v[h] * tanh(qh[q,h] + kh[k,h])
    attn = softmax_k(mask(scores));  out = attn @ values

Device-side layout: everything is computed in the transposed [k, q]
orientation so the exp output feeds the attention*values matmul directly
(k on partitions = contraction dim) with zero transposes of the big
intermediates. Two k's are packed per 128-partition tile (H=64), so the
tanh feature tile for "k-pair" i is
    feat[64*j + h, q] = tanh(qh[q,h] + kh[2i+j, h]),  j in {0,1}
built by a DVE per-partition-scalar add (qh2 + khp[:, i], fp16 for the
4x DVE perf mode) and one big ACT Tanh (the 33.5M-element ACT work is
the kernel's hard floor, ~218us/core). A block-diagonal fp16 stationary
matrix per pair reduces over h on the PE (full rate) accumulating
transposed fp32 scores [128 k, 512 q] per k-tile in PSUM. The fp16
rounding (11-bit mantissa, same class as TF32) costs ~2e-4 relative
error on the output. Masking rides for free as the per-partition bias
of the Exp activation (bias 0 or -1e6; exp -> exact 0), so no max
subtraction is needed (|scores| <= ||w_v||_1 ~ 7, exp never overflows).
Softmax normalization is deferred: sums over k via a ones-vector matmul,
reciprocal on the tiny [Q] vector, applied after the final transpose.
The first/last k-tiles use graduated chunk sizes so the ACT pipeline
ramps with the DMA prologue and drains into the epilogue.
"""

import numpy as np

B, Q, K = 8, 512, 1024
DQ, DK, DV, H = 128, 128, 128, 64
MASK_VAL = -1000000.0

N_CORES = 8
KT = K // 128          # 8 k-tiles of 128 keys
PAIRS = K // 2         # 512 k-pairs
PPC = 32               # pairs per tanh chunk
CHUNK_FD = PPC * Q     # 16384
PAIRS_PER_KT = 64      # pairs per k-tile
CHUNKS_PER_KT = PAIRS_PER_KT // PPC  # 2
QT = Q // 128          # 4 q-tiles

_CACHE = {}


def _build_nc():
    import concourse.bacc as bacc
    import concourse.tile as tile
    from concourse import mybir

    f32 = mybir.dt.float32
    f16 = mybir.dt.float16

    nc = bacc.Bacc("TRN2", target_bir_lowering=False, debug=False,
                   num_devices=N_CORES)

    qh2_d = nc.dram_tensor("qh2", [128, Q], f16, kind="ExternalInput")
    khp_d = nc.dram_tensor("khp", [128, PAIRS], f32, kind="ExternalInput")
    vals_d = nc.dram_tensor("vals", [K, DV], f32, kind="ExternalInput")
    mask_d = nc.dram_tensor("maskT", [128, KT], f32, kind="ExternalInput")
    wvb_d = nc.dram_tensor("wvb", [128, PAIRS_PER_KT * 128], f16,
                           kind="ExternalInput")
    ident_d = nc.dram_tensor("ident", [128, 128], f32, kind="ExternalInput")
    out_d = nc.dram_tensor("out", [Q, DV], f32, kind="ExternalOutput")

    Tanh = mybir.ActivationFunctionType.Tanh
    Exp = mybir.ActivationFunctionType.Exp

    with tile.TileContext(nc) as tc:
        with (
            tc.tile_pool(name="const", bufs=1) as cpool,
            tc.tile_pool(name="attn", bufs=1) as apool,
            tc.tile_pool(name="fin", bufs=2) as fin_pool,
            tc.tile_pool(name="fout", bufs=2) as fout_pool,
            tc.tile_pool(name="small", bufs=1) as spool,
            tc.tile_pool(name="osb", bufs=2) as opool,
            tc.tile_pool(name="ps_scores", bufs=2, space="PSUM") as ps_s,
            tc.tile_pool(name="ps_sums", bufs=1, space="PSUM") as ps_sum,
            tc.tile_pool(name="ps_outT", bufs=1, space="PSUM") as ps_o,
            tc.tile_pool(name="ps_rt", bufs=1, space="PSUM") as ps_rt,
            tc.tile_pool(name="ps_oq", bufs=2, space="PSUM") as ps_oq,
        ):
            # ---- load constants/inputs ----
            # order matters: qh2/khp feed the first DVE adds; wvb_a feeds the
            # first chunk's score matmuls; everything else is needed later.
            qh2 = cpool.tile([128, Q], f16)
            nc.sync.dma_start(qh2[:], qh2_d[:])
            khp = cpool.tile([128, PAIRS], f32)
            nc.sync.dma_start(khp[:], khp_d[:])
            wvb_a = cpool.tile([128, PPC * 128], f16)
            nc.sync.dma_start(wvb_a[:], wvb_d[:, 0:PPC * 128])
            wvb_b = cpool.tile([128, (PAIRS_PER_KT - PPC) * 128], f16)
            nc.sync.dma_start(wvb_b[:], wvb_d[:, PPC * 128:])
            maskT = cpool.tile([128, KT], f32)
            nc.sync.dma_start(maskT[:], mask_d[:])
            vals = cpool.tile([128, KT * 128], f32)
            for t in range(KT):
                nc.sync.dma_start(vals[:, t * 128:(t + 1) * 128],
                                  vals_d[t * 128:(t + 1) * 128, :])
            ident = cpool.tile([128, 128], f32)
            nc.sync.dma_start(ident[:], ident_d[:])
            ones_col = cpool.tile([128, 1], f32)
            nc.vector.memset(ones_col[:], 1.0)

            def wvb_slice(ii):
                if ii < PPC:
                    return wvb_a[:, ii * 128:(ii + 1) * 128]
                return wvb_b[:, (ii - PPC) * 128:(ii - PPC + 1) * 128]

            attn = apool.tile([128, KT * Q], f32)
            ps_sums = ps_sum.tile([1, Q], f32)
            ps_out = ps_o.tile([128, Q], f32)

            def sums_av(t):
                nc.tensor.matmul(ps_sums[:], ones_col[:],
                                 attn[:, t * Q:(t + 1) * Q],
                                 start=(t == 0), stop=(t == KT - 1))
                nc.tensor.matmul(ps_out[:],
                                 vals[:, t * 128:(t + 1) * 128],
                                 attn[:, t * Q:(t + 1) * Q],
                                 start=(t == 0), stop=(t == KT - 1))

            # ---- main loop: tanh features + score reduction ----
            # The exp for k-tile t-1 is emitted one chunk into k-tile t, and
            # its sums/attn@values matmuls one chunk later still, so neither
            # the in-order ACT stream nor the PE ever waits on a just-closed
            # score accumulation group.
            def chunk_plan(t):
                # Small chunks at the very start (first tanh launches early,
                # right after the qh2/khp DMAs) and at the very end (the
                # final exp waits on only a few trailing score matmuls).
                if t == 0:
                    return [8, 8, 8, 16, 24]
                if t == KT - 1:
                    return [24, 16, 8, 8, 4, 4]
                return [PPC] * CHUNKS_PER_KT

            prev_ps = None
            for t in range(KT):
                ps = ps_s.tile([128, Q], f32)
                ii = 0
                for c, width in enumerate(chunk_plan(t)):
                    fin = fin_pool.tile([128, width * Q], f16)
                    for j in range(width):
                        pair = t * PAIRS_PER_KT + ii + j
                        nc.vector.tensor_scalar_add(
                            fin[:, j * Q:(j + 1) * Q], qh2[:],
                            khp[:, pair:pair + 1])
                    fout = fout_pool.tile([128, width * Q], f16)
                    nc.scalar.activation(fout[:], fin[:], Tanh)
                    for j in range(width):
                        nc.tensor.matmul(
                            ps[:],
                            wvb_slice(ii + j),
                            fout[:, j * Q:(j + 1) * Q],
                            start=(ii + j == 0),
                            stop=(ii + j == PAIRS_PER_KT - 1))
                    ii += width
                    if c == 0 and t > 0:
                        nc.scalar.activation(attn[:, (t - 1) * Q:t * Q],
                                             prev_ps[:], Exp,
                                             bias=maskT[:, t - 1:t])
                    if c == 1 and t > 0:
                        sums_av(t - 1)
                prev_ps = ps
            nc.scalar.activation(attn[:, (KT - 1) * Q:KT * Q], prev_ps[:],
                                 Exp, bias=maskT[:, KT - 1:KT])
            sums_av(KT - 1)

            # ---- normalize + transpose back to [q, v] ----
            sums_sb = spool.tile([1, Q], f32)
            nc.vector.tensor_copy(sums_sb[:], ps_sums[:])
            rt = ps_rt.tile([128, QT], f32)
            for qt in range(QT):
                nc.tensor.transpose(rt[:, qt:qt + 1],
                                    sums_sb[0:1, qt * 128:(qt + 1) * 128],
                                    ident[0:1, 0:1],
                                    )
            recip = spool.tile([128, QT], f32)
            nc.vector.reciprocal(recip[:], rt[:])

            outT = spool.tile([128, Q], f32)
            nc.vector.tensor_copy(outT[:], ps_out[:])
            for qt in range(QT):
                oq = ps_oq.tile([128, 128], f32)
                nc.tensor.transpose(oq[:], outT[:, qt * 128:(qt + 1) * 128],
                                    ident[:])
                osb = opool.tile([128, 128], f32)
                nc.vector.tensor_scalar_mul(osb[:], oq[:],
                                            recip[:, qt:qt + 1])
                nc.sync.dma_start(out_d[qt * 128:(qt + 1) * 128, :], osb[:])

    nc.compile()
    return nc


def _get_nc():
    if "nc" not in _CACHE:
        _CACHE["nc"] = _build_nc()
    return _CACHE["nc"]


def _host_prep(queries, keys, values, valid_lens, W_q, W_k, w_v):
    """Build the per-core input maps (shard over batch)."""
    queries = np.asarray(queries, dtype=np.float32)
    keys = np.asarray(keys, dtype=np.float32)
    values = np.asarray(values, dtype=np.float32)
    valid_lens = np.asarray(valid_lens)
    W_q = np.asarray(W_q, dtype=np.float32)
    W_k = np.asarray(W_k, dtype=np.float32)
    w_v = np.asarray(w_v, dtype=np.float32)

    # shared across cores
    wvb = np.zeros((128, PAIRS_PER_KT * 128), dtype=np.float16)
    w_v_h = w_v.astype(np.float16)
    for ii in range(PAIRS_PER_KT):
        wvb[0:H, ii * 128 + 2 * ii] = w_v_h
        wvb[H:128, ii * 128 + 2 * ii + 1] = w_v_h
    ident = np.eye(128, dtype=np.float32)
    karr = np.arange(K, dtype=np.int64).reshape(KT, 128).T  # [128, KT]

    in_maps = []
    for b in range(B):
        qh = queries[b] @ W_q                      # [Q, H]
        kh = keys[b] @ W_k                         # [K, H]
        qh2 = np.concatenate([qh.T, qh.T], axis=0).astype(np.float16)  # [128, Q]
        khT3 = kh.T.reshape(H, PAIRS, 2)
        khp = np.concatenate([khT3[:, :, 0], khT3[:, :, 1]], axis=0)  # [128, PAIRS]
        vl = int(valid_lens[b])
        maskT = np.where(karr < vl, 0.0, MASK_VAL).astype(np.float32)
        in_maps.append({
            "qh2": np.ascontiguousarray(qh2),
            "khp": np.ascontiguousarray(khp),
            "vals": np.ascontiguousarray(values[b]),
            "maskT": np.ascontiguousarray(maskT),
            "wvb": wvb,
            "ident": ident,
        })
    return in_maps


def kernel(queries, keys, values, valid_lens, W_q, W_k, w_v):
    from concourse.bass_utils import run_bass_kernel_spmd

    nc = _get_nc()
    in_maps = _host_prep(queries, keys, values, valid_lens, W_q, W_k, w_v)
    res = run_bass_kernel_spmd(nc, in_maps, list(range(N_CORES)))
    out = np.stack([res.results[i]["out"] for i in range(N_CORES)], axis=0)
    return out.astype(np.float32)


if __name__ == "__main__":
    rng = np.random.default_rng(0)
    inputs = {
        "queries": rng.standard_normal((B, Q, DQ), dtype=np.float32),
        "keys": rng.standard_normal((B, K, DK), dtype=np.float32),
        "values": rng.standard_normal((B, K, DV), dtype=np.float32),
        "valid_lens": rng.integers(1, K + 1, size=(B,), dtype=np.int32),
        "W_q": (rng.standard_normal((DQ, H)) / np.sqrt(DQ)).astype(np.float32),
        "W_k": (rng.standard_normal((DK, H)) / np.sqrt(DK)).astype(np.float32),
        "w_v": (rng.standard_normal((H,)) / np.sqrt(H)).astype(np.float32),
    }
    out = kernel(**inputs)
    print("out", out.shape, out.dtype)
